# revision 14
# baseline (speedup 1.0000x reference)
"""Trainium2 Bass kernel for nn_EnhancedGAT (3-layer GATv2, N=10000, E=160000).

Strategy (8 NeuronCores, SPMD), v2 (bf16):
  - Destination-partition the graph: each core owns 1250 dst nodes (padded to
    1280 = 10 windows x 128). Edges (incl. self-loops) are sorted by dst on the
    host and bucketed into (core, window); each window's edge list is padded to
    T_w tiles of 128 edges.
  - Segment softmax without segment-max (logits are tiny):
        out = (sum_e exp(l_e) * xl[src_e]) / (sum_e exp(l_e) + eps)
  - All matmul operands are bf16 (1 cyc/row on the PE vs 4 for fp32); PSUM
    accumulation stays fp32. leaky_relu is a single Prelu activation
    (alpha=0.2). Logit dot = tensor_tensor mult + tensor_reduce (bf16, 2x DVE).
  - Layer 1 "G-trick": aggregate w-weighted one-hot @ raw x features per
    window, multiply by Wl1 once per window. x[src] gather is done on the
    HOST (x is an input) and streamed as a dense bf16 table.
  - Layers 2/3 gather from bf16 xl tables (local compute + AllGather, bf16),
    ONE CounterMachine dma_gather per window (not per tile), reused by both
    passes.
  - Wl2/Wr2 resident in SBUF as bf16 (no per-window reloads).
  - One-hots generated on-chip (iota is_equal), cached per window, transposed
    on the PE in bf16.
"""
import os
import numpy as np
import ml_dtypes

import concourse.bass as bass
import concourse.bacc as bacc
import concourse.mybir as mybir
import concourse.tile as tile
from concourse.bass_utils import run_bass_kernel_spmd
from concourse.masks import make_identity

F32 = mybir.dt.float32
BF16 = mybir.dt.bfloat16
I32 = mybir.dt.int32
I16 = mybir.dt.int16
AF = mybir.ActivationFunctionType
OP = mybir.AluOpType
bfnp = ml_dtypes.bfloat16

NC_CORES = 8
N = 10000
ND = 128
ED = 32
PER = N // NC_CORES          # 1250
NPAD = 1280
W = NPAD // 128              # 10 windows
EPS = 1e-16

_DEBUG = bool(int(os.environ.get("GAT_DEBUG", "0")))
_WLIM = int(os.environ.get("GAT_WLIM", str(W)))    # windows to emit (dev only)
_LAYERS = int(os.environ.get("GAT_LAYERS", "3"))   # dev only
_NOCOLL = bool(int(os.environ.get("GAT_NOCOLL", "0")))  # sim only
_NOCOLL3 = bool(int(os.environ.get("GAT_NOCOLL3", "0")))  # triage: stub AG#2
_REPEAT = int(os.environ.get("GAT_REPEAT", "1"))  # timing: run pipeline R times


# ----------------------------------------------------------------------------
# host-side prep
# ----------------------------------------------------------------------------

def _host_prep(x, edge_index, edge_attr):
    src = np.concatenate([edge_index[0], np.arange(N)]).astype(np.int64)
    dst = np.concatenate([edge_index[1], np.arange(N)]).astype(np.int64)
    ea = np.concatenate(
        [edge_attr, np.tile(edge_attr.mean(0), (N, 1))], axis=0
    ).astype(np.float32)

    core_of = dst // PER
    loc = dst % PER
    win_of = loc // 128
    dst_rel = (loc % 128).astype(np.float32)

    key = core_of * W + win_of
    order = np.argsort(key, kind="stable")
    counts = np.bincount(key[order], minlength=NC_CORES * W)
    T_w = int(np.ceil(counts.max() / 128))
    EPW = T_w * 128

    starts = np.zeros(NC_CORES * W, np.int64)
    starts[1:] = np.cumsum(counts)[:-1]

    x_bf = x.astype(bfnp)
    src23 = np.zeros((NC_CORES, W, EPW), np.int64)
    drel = np.full((NC_CORES, W, EPW), -1.0, np.float32)
    eaT = np.zeros((NC_CORES, W, ED, EPW), np.float32)
    xga = np.zeros((NC_CORES, W * 128, EPW), bfnp)   # [c, w*128+p, t*128+f]
    for c in range(NC_CORES):
        for w in range(W):
            k = int(counts[c * W + w])
            m = order[starts[c * W + w]: starts[c * W + w] + k]
            s = np.zeros(EPW, np.int64)
            s[:k] = src[m]
            src23[c, w] = (s // PER) * NPAD + (s % PER)
            drel[c, w, :k] = dst_rel[m]
            eaT[c, w, :, :k] = ea[m].T
            g = x_bf[s]                                  # [EPW, 128]
            xga[c, w * 128:(w + 1) * 128] = (
                g.reshape(T_w, 128, ND).transpose(1, 0, 2).reshape(128, EPW))
    return T_w, EPW, src23, drel, eaT, xga


def _pad_own(a, c):
    out = np.zeros((NPAD,) + a.shape[1:], a.dtype)
    out[:PER] = a[c * PER: (c + 1) * PER]
    return out


def _chunks_for_rhs(Wm):
    """[K, F] weight -> [128, (K//128)*F]: chunk k at cols [k*F:(k+1)*F]."""
    K, F = Wm.shape
    assert K % 128 == 0
    return np.ascontiguousarray(
        Wm.reshape(K // 128, 128, F).transpose(1, 0, 2).reshape(128, -1)
    )


def _idx16_wrap(idx, EPW):
    """[W, EPW] int -> [W*128, EPW//16] int16 in dma_gather layout:
    wrapped in 16 partitions, replicated across the 8 GpSimd core banks."""
    Wn = idx.shape[0]
    wrap = idx.reshape(Wn, EPW // 16, 16).transpose(0, 2, 1)   # [W, 16, EPW//16]
    out = np.tile(wrap, (1, 8, 1)).astype(np.int16)            # [W, 128, EPW//16]
    return np.ascontiguousarray(out.reshape(Wn * 128, EPW // 16))


# ----------------------------------------------------------------------------
# bass program
# ----------------------------------------------------------------------------

def _build_program(T_w, repeat=None):
    repeat = _REPEAT if repeat is None else repeat
    EPW = T_w * 128
    EPW16 = EPW // 16
    nc = bacc.Bacc("TRN2", target_bir_lowering=False, debug=False,
                   enable_asserts=False, num_devices=NC_CORES)

    def din(name, shape, dt=BF16):
        return nc.dram_tensor(name, shape, dt, kind="ExternalInput")

    x_ownT = din("x_ownT", [ND, NPAD])
    xga_d = din("xga_t", [W * 128, EPW])
    src23_d = din("src23i", [W * 128, T_w], I32)
    drel_d = din("drel", [W * 128, T_w], F32)
    eaT_d = din("eaT", [W * ED, EPW])
    iotar_d = din("iotarb", [128, 128])

    Wl1_d = din("Wl1", [128, 1024])
    Wr1_d = din("Wr1", [128, 1024])
    Wres_d = din("Wres", [128, 1024])
    We1_d = din("We1", [ED, 1024])
    att1_d = din("att1b", [128, 1024])
    Wl2_d = din("Wl2c", [128, 8 * 512])
    Wr2_d = din("Wr2c", [128, 8 * 512])
    We2_d = din("We2", [ED, 512])
    att2_d = din("att2b", [128, 512])
    Wl3_d = din("Wl3", [128, 128])
    Wr3_d = din("Wr3", [128, 128])
    We3_d = din("We3", [ED, 128])
    att3_d = din("att3b", [128, 128])
    Wc1_d = din("Wc1", [128, 64])
    Wc2_d = din("Wc2", [64, 3])
    biasr1_d = din("biasr1", [1, 1024])
    const1_d = din("const1", [1, 1024])
    biasr2_d = din("biasr2", [1, 512])
    const2_d = din("const2b", [128, 128], F32)
    biasr3_d = din("biasr3", [1, 128])
    const3_d = din("const3b", [128, 128], F32)
    bc1_d = din("bc1", [1, 64])
    bc2_d = din("bc2", [1, 3])

    out_d = nc.dram_tensor("out_o", [NPAD, 3], F32, kind="ExternalOutput")
    if _DEBUG:
        h1_dbg = nc.dram_tensor("h1_dbg", [NPAD, 1024], F32, kind="ExternalOutput")
        h2_dbg = nc.dram_tensor("h2_dbg", [NPAD, 128], F32, kind="ExternalOutput")
        h3_dbg = nc.dram_tensor("h3_dbg", [NPAD, 128], F32, kind="ExternalOutput")

    with tile.TileContext(nc) as tc:
        with tc.tile_pool(name="wp", bufs=1) as wp, \
             tc.tile_pool(name="slab", bufs=1) as slab, \
             tc.tile_pool(name="io", bufs=2) as io, \
             tc.tile_pool(name="io3", bufs=3) as io3, \
             tc.tile_pool(name="fat", bufs=2) as fat, \
             tc.tile_pool(name="big", bufs=2) as big, \
             tc.tile_pool(name="psS", bufs=3, space="PSUM") as psS, \
             tc.tile_pool(name="psG", bufs=1, space="PSUM") as psG, \
             tc.tile_pool(name="psW", bufs=1, space="PSUM") as psW, \
             tc.tile_pool(name="psT", bufs=2, space="PSUM") as psT, \
             tc.tile_pool(name="dram", bufs=1, space="DRAM") as dr:

            # ---------- resident constants ----------
            def load(dram_t, shape, name, dt=BF16):
                t = wp.tile(shape, dt, name=name, tag=name)
                nc.sync.dma_start(out=t[:], in_=dram_t.ap())
                return t

            Wl1 = load(Wl1_d, [128, 1024], "Wl1")
            Wr1 = load(Wr1_d, [128, 1024], "Wr1")
            Wres = load(Wres_d, [128, 1024], "Wres")
            We1 = load(We1_d, [ED, 1024], "We1")
            att1b = load(att1_d, [128, 1024], "att1b")
            Wl2c = load(Wl2_d, [128, 8 * 512], "Wl2c")
            Wr2c = load(Wr2_d, [128, 8 * 512], "Wr2c")
            We2 = load(We2_d, [ED, 512], "We2")
            att2b = load(att2_d, [128, 512], "att2b")
            Wl3 = load(Wl3_d, [128, 128], "Wl3")
            Wr3 = load(Wr3_d, [128, 128], "Wr3")
            We3 = load(We3_d, [ED, 128], "We3")
            att3b = load(att3_d, [128, 128], "att3b")
            Wc1 = load(Wc1_d, [128, 64], "Wc1")
            Wc2 = load(Wc2_d, [64, 3], "Wc2")
            biasr1 = load(biasr1_d, [1, 1024], "biasr1")
            const1 = load(const1_d, [1, 1024], "const1")
            biasr2 = load(biasr2_d, [1, 512], "biasr2")
            const2b = load(const2_d, [128, 128], "const2b", F32)
            biasr3 = load(biasr3_d, [1, 128], "biasr3")
            const3b = load(const3_d, [128, 128], "const3b", F32)
            bc1 = load(bc1_d, [1, 64], "bc1")
            bc2 = load(bc2_d, [1, 3], "bc2")
            iotarb = load(iotar_d, [128, 128], "iotarb")

            ident = wp.tile([128, 128], F32, name="ident", tag="ident")
            make_identity(nc, ident[:])
            identb = wp.tile([128, 128], BF16, name="identb", tag="identb")
            nc.vector.tensor_copy(out=identb[:], in_=ident[:])
            ones1 = wp.tile([1, 128], BF16, name="ones1", tag="ones1")
            nc.vector.memset(ones1[:], 1.0)

            xr2_own = slab.tile([128, W * 512], BF16, name="xr2_own", tag="xr2_own")
            xr3_own = slab.tile([128, W * 128], BF16, name="xr3_own", tag="xr3_own")

            # ---------- helpers ----------
            def onehot_to(ohs, drel_w, t, lidx, w):
                """ohs[:, t*128:(t+1)*128] = (iota == drel[e]) in bf16."""
                nc.vector.tensor_scalar(
                    out=ohs[:, t * 128:(t + 1) * 128], in0=iotarb[:],
                    scalar1=drel_w[:, t:t + 1], scalar2=None, op0=OP.is_equal)

            def transposeb(src_sl, name, copy_eng="act"):
                """bf16 PE transpose + PSUM->SBUF copy."""
                pt = psT.tile([128, 128], BF16, name=f"pt{name}", tag="pt")
                nc.tensor.transpose(out=pt[:], in_=src_sl, identity=identb[:])
                out = io3.tile([128, 128], BF16, name=f"T{name}", tag="Tt")
                if copy_eng == "act":
                    nc.scalar.copy(out=out[:], in_=pt[:])
                else:
                    nc.vector.tensor_copy(out=out[:], in_=pt[:])
                return out

            def window_meta(w, lidx):
                drel_w = io.tile([128, T_w], F32, name=f"drel{lidx}_{w}", tag="drel")
                nc.sync.dma_start(out=drel_w[:], in_=drel_d.ap()[w * 128:(w + 1) * 128, :])
                ea_w = big.tile([ED, EPW], BF16, name=f"ea{lidx}_{w}", tag="ea")
                nc.sync.dma_start(out=ea_w[:], in_=eaT_d.ap()[w * ED:(w + 1) * ED, :])
                return drel_w, ea_w

            def logits_we(s, attb, H, we_out, w, t, lidx):
                """we_out = exp(per-head <s, att>), s/att bf16."""
                q = fat.tile([128, H * 128], BF16, name=f"q{lidx}_{w}_{t}", tag="q")
                nc.vector.tensor_tensor(out=q[:], in0=s[:], in1=attb[:, :H * 128],
                                        op=OP.mult)
                lg = io.tile([128, H], BF16, name=f"lg{lidx}_{w}_{t}", tag="lg")
                uv = q[:].rearrange("p (h c) -> p h c", h=H) if H > 1 else q[:]
                with nc.allow_low_precision(reason="logits tolerate bf16"):
                    nc.vector.tensor_reduce(out=lg[:], in_=uv,
                                            axis=mybir.AxisListType.X, op=OP.add)
                nc.scalar.activation(out=we_out, in_=lg[:], func=AF.Exp)

            def rz_from(ps_z, H, w, lidx, quarter=False):
                zt = io.tile([128, H], F32, name=f"zt{lidx}_{w}", tag="zt")
                nc.vector.tensor_scalar(out=zt[:], in0=ps_z, scalar1=EPS,
                                        scalar2=None, op0=OP.add)
                rz = io.tile([128, H], F32, name=f"rz{lidx}_{w}", tag="rz")
                nc.vector.reciprocal(out=rz[:], in_=zt[:])
                if quarter:
                    nc.vector.tensor_scalar(out=rz[:], in0=rz[:], scalar1=0.25,
                                            scalar2=None, op0=OP.mult)
                return rz

            def elu_of(a, F, w, lidx):
                """h = elu(a) = relu(a) + min(exp(a)-1, 0). f32 in/out."""
                ex = io.tile([128, F], F32, name=f"ex{lidx}_{w}", tag="ex")
                nc.scalar.activation(out=ex[:], in_=a[:], func=AF.Exp)
                em = io.tile([128, F], F32, name=f"em{lidx}_{w}", tag="em")
                nc.vector.tensor_scalar(out=em[:], in0=ex[:], scalar1=1.0, scalar2=0.0,
                                        op0=OP.subtract, op1=OP.min)
                r = io.tile([128, F], F32, name=f"r{lidx}_{w}", tag="r")
                nc.scalar.activation(out=r[:], in_=a[:], func=AF.Relu)
                h = io.tile([128, F], F32, name=f"h{lidx}_{w}", tag="helu")
                nc.vector.tensor_tensor(out=h[:], in0=r[:], in1=em[:], op=OP.add)
                return h

            for rep in range(repeat):
                xl2_bounce = dr.tile([NPAD, 512], BF16, name=f"xl2_bounce_{rep}",
                                     tag=f"x2b{rep}")
                xl2_full = dr.tile([NC_CORES * NPAD, 512], BF16,
                                   name=f"xl2_full_{rep}", addr_space="Shared",
                                   tag=f"x2f{rep}")
                xl3_bounce = dr.tile([NPAD, 128], F32, name=f"xl3_bounce_{rep}",
                                     tag=f"x3b{rep}")
                xl3_full = dr.tile([NC_CORES * NPAD, 128], F32,
                                   name=f"xl3_full_{rep}", addr_space="Shared",
                                   tag=f"x3f{rep}")
                # =========================================================
                # LAYER 1 (H=8, G-trick, host-gathered x)
                # =========================================================
                for w in range(_WLIM):
                    drel_w, ea_w = window_meta(w, 1)
                    xo_w = io.tile([128, 128], BF16, name=f"xo_{w}", tag="xo")
                    nc.sync.dma_start(out=xo_w[:], in_=x_ownT.ap()[:, w * 128:(w + 1) * 128])
                    xga = big.tile([128, EPW], BF16, name=f"xga1_{w}", tag="gxa")
                    nc.sync.dma_start(out=xga[:], in_=xga_d.ap()[w * 128:(w + 1) * 128, :])
                    ohs = big.tile([128, EPW], BF16, name=f"ohs1_{w}", tag="ohs")
                    we_w = io.tile([128, T_w * 8], F32, name=f"wew1_{w}", tag="wew")

                    # xr1 window: x_win @ Wr1 + (br1+bl1), bf16 out
                    xr1_w = big.tile([128, 1024], BF16, name=f"xr1_{w}", tag="xr1")
                    for j in range(2):
                        sl = slice(j * 512, (j + 1) * 512)
                        ps_xr = psS.tile([128, 512], F32, name=f"psxr1_{w}_{j}", tag="S")
                        nc.tensor.matmul(out=ps_xr[:], lhsT=xo_w[:],
                                         rhs=Wr1[:, sl], start=True, stop=False)
                        nc.tensor.matmul(out=ps_xr[:], lhsT=ones1[:],
                                         rhs=biasr1[:, sl], start=False, stop=True)
                        nc.scalar.copy(out=xr1_w[:, sl], in_=ps_xr[:])

                    # ---- pass A: edge scores ----
                    for t in range(T_w):
                        onehot_to(ohs, drel_w, t, 1, w)
                        ohT = transposeb(ohs[:, t * 128:(t + 1) * 128],
                                         f"o1_{w}_{t}", "act")
                        xgT = transposeb(xga[:, t * 128:(t + 1) * 128],
                                         f"x1_{w}_{t}", "vec")
                        s = fat.tile([128, 1024], BF16, name=f"s1_{w}_{t}", tag="s")
                        for j in range(2):
                            sl = slice(j * 512, (j + 1) * 512)
                            ps_S = psS.tile([128, 512], F32, name=f"psS1_{w}_{t}_{j}",
                                            tag="S")
                            nc.tensor.matmul(out=ps_S[:], lhsT=xgT[:],
                                             rhs=Wl1[:, sl], start=True, stop=False)
                            nc.tensor.matmul(out=ps_S[:],
                                             lhsT=ea_w[:, t * 128:(t + 1) * 128],
                                             rhs=We1[:, sl], start=False, stop=False)
                            nc.tensor.matmul(out=ps_S[:], lhsT=ohT[:],
                                             rhs=xr1_w[:, sl], start=False, stop=True)
                            nc.scalar.activation(out=s[:, sl], in_=ps_S[:],
                                                 func=AF.Prelu, alpha=0.2)
                        logits_we(s, att1b, 8, we_w[:, t * 8:(t + 1) * 8], w, t, 1)

                    # ---- pass B: weighted scatter (G-trick) ----
                    ps_G = psG.tile([128, 1024], F32, name=f"psG_{w}", tag="G")
                    ps_z = psW.tile([128, 8], F32, name=f"psz1_{w}", tag="Wz")
                    for t in range(T_w):
                        re = fat.tile([128, 1032], BF16, name=f"re_{w}_{t}", tag="re")
                        for h in range(8):
                            nc.vector.tensor_scalar(
                                out=re[:, h * 128:(h + 1) * 128],
                                in0=xga[:, t * 128:(t + 1) * 128],
                                scalar1=we_w[:, t * 8 + h:t * 8 + h + 1],
                                scalar2=None, op0=OP.mult)
                        nc.vector.tensor_copy(out=re[:, 1024:1032],
                                              in_=we_w[:, t * 8:(t + 1) * 8])
                        oh_sl = ohs[:, t * 128:(t + 1) * 128]
                        for j in range(2):
                            nc.tensor.matmul(out=ps_G[:, j * 512:(j + 1) * 512],
                                             lhsT=oh_sl, rhs=re[:, j * 512:(j + 1) * 512],
                                             start=(t == 0), stop=(t == T_w - 1))
                        nc.tensor.matmul(out=ps_z[:], lhsT=oh_sl,
                                         rhs=re[:, 1024:1032],
                                         start=(t == 0), stop=(t == T_w - 1))

                    # ---- window flush ----
                    rz = rz_from(ps_z[:], 8, w, 1)
                    gn = fat.tile([128, 1024], BF16, name=f"gn_{w}", tag="s")
                    for h in range(8):
                        nc.vector.tensor_scalar(out=gn[:, h * 128:(h + 1) * 128],
                                                in0=ps_G[:, h * 128:(h + 1) * 128],
                                                scalar1=rz[:, h:h + 1],
                                                scalar2=None, op0=OP.mult)
                    gnT = fat.tile([128, 1024], BF16, name=f"gnT_{w}", tag="q")
                    for h in range(8):
                        pt = psT.tile([128, 128], BF16, name=f"ptg_{w}_{h}", tag="pt")
                        nc.tensor.transpose(out=pt[:], in_=gn[:, h * 128:(h + 1) * 128],
                                            identity=identb[:])
                        eng = nc.scalar.copy if h % 2 else (
                            lambda out, in_: nc.vector.tensor_copy(out=out, in_=in_))
                        eng(out=gnT[:, h * 128:(h + 1) * 128], in_=pt[:])
                    h1_w = fat.tile([128, 1024], BF16, name=f"h1_{w}", tag="re")
                    for j in range(2):
                        sl = slice(j * 512, (j + 1) * 512)
                        ps_O = psS.tile([128, 512], F32, name=f"psO1_{w}_{j}", tag="S")
                        for h in range(4 * j, 4 * j + 4):
                            nc.tensor.matmul(out=ps_O[:, (h % 4) * 128:(h % 4 + 1) * 128],
                                             lhsT=gnT[:, h * 128:(h + 1) * 128],
                                             rhs=Wl1[:, h * 128:(h + 1) * 128],
                                             start=(h % 4 == 0), stop=False)
                        nc.tensor.matmul(out=ps_O[:], lhsT=xo_w[:],
                                         rhs=Wres[:, sl], start=False, stop=False)
                        nc.tensor.matmul(out=ps_O[:], lhsT=ones1[:],
                                         rhs=const1[:, sl], start=False, stop=True)
                        nc.scalar.activation(out=h1_w[:, sl], in_=ps_O[:],
                                             func=AF.Prelu, alpha=0.2)
                    if _DEBUG:
                        h1f = fat.tile([128, 1024], F32, name=f"h1f_{w}", tag="h1f")
                        nc.vector.tensor_copy(out=h1f[:], in_=h1_w[:])
                        nc.sync.dma_start(out=h1_dbg.ap()[w * 128:(w + 1) * 128, :],
                                          in_=h1f[:])
                    h1T = fat.tile([128, 1024], BF16, name=f"h1T_{w}", tag="q")
                    for h in range(8):
                        pt = psT.tile([128, 128], BF16, name=f"pth_{w}_{h}", tag="pt")
                        nc.tensor.transpose(out=pt[:], in_=h1_w[:, h * 128:(h + 1) * 128],
                                            identity=identb[:])
                        eng = nc.scalar.copy if h % 2 else (
                            lambda out, in_: nc.vector.tensor_copy(out=out, in_=in_))
                        eng(out=h1T[:, h * 128:(h + 1) * 128], in_=pt[:])
                    ps_x2 = psS.tile([128, 512], F32, name=f"psx2_{w}", tag="S")
                    for k in range(8):
                        nc.tensor.matmul(out=ps_x2[:], lhsT=h1T[:, k * 128:(k + 1) * 128],
                                         rhs=Wl2c[:, k * 512:(k + 1) * 512],
                                         start=(k == 0), stop=(k == 7))
                    xl2_w = io.tile([128, 512], BF16, name=f"xl2_{w}", tag="xl2")
                    nc.scalar.copy(out=xl2_w[:], in_=ps_x2[:])
                    nc.sync.dma_start(out=xl2_bounce[w * 128:(w + 1) * 128, :], in_=xl2_w[:])
                    ps_r2 = psS.tile([128, 512], F32, name=f"psr2_{w}", tag="S")
                    for k in range(8):
                        nc.tensor.matmul(out=ps_r2[:], lhsT=h1T[:, k * 128:(k + 1) * 128],
                                         rhs=Wr2c[:, k * 512:(k + 1) * 512],
                                         start=(k == 0), stop=False)
                    nc.tensor.matmul(out=ps_r2[:], lhsT=ones1[:], rhs=biasr2[:],
                                     start=False, stop=True)
                    nc.scalar.copy(out=xr2_own[:, w * 512:(w + 1) * 512], in_=ps_r2[:])

                # =========================================================
                # LAYER 2 (H=4, dma_gather of bf16 xl2)
                # =========================================================
                if _LAYERS >= 2:
                    if _NOCOLL:
                        nc.sync.dma_start(out=xl2_full[0:NPAD, :], in_=xl2_bounce[:])
                    else:
                        nc.gpsimd.collective_compute(
                            "AllGather", OP.bypass,
                            replica_groups=[list(range(NC_CORES))],
                            ins=[xl2_bounce[:]], outs=[xl2_full[:]])

                    for w in range(_WLIM):
                        drel_w, ea_w = window_meta(w, 2)
                        idx_w = io.tile([128, T_w], I32, name=f"ix2_{w}", tag="ix")
                        nc.sync.dma_start(out=idx_w[:],
                                          in_=src23_d.ap()[w * 128:(w + 1) * 128, :])
                        xlg = big.tile([128, T_w * 512], BF16, name=f"xlg_{w}", tag="xlg")
                        for tt in range(T_w):
                            nc.gpsimd.indirect_dma_start(
                                out=xlg[:, tt * 512:(tt + 1) * 512], out_offset=None,
                                in_=xl2_full[:],
                                in_offset=bass.IndirectOffsetOnAxis(
                                    ap=idx_w[:, tt:tt + 1], axis=0))
                        ohs = big.tile([128, EPW], BF16, name=f"ohs2_{w}", tag="ohs")
                        we_w = io.tile([128, T_w * 4], F32, name=f"wew2_{w}", tag="wew")

                        # ---- pass A ----
                        for t in range(T_w):
                            onehot_to(ohs, drel_w, t, 2, w)
                            ohT = transposeb(ohs[:, t * 128:(t + 1) * 128],
                                             f"o2_{w}_{t}", "act")
                            ps_B = psS.tile([128, 512], F32, name=f"psB2_{w}_{t}", tag="S")
                            nc.tensor.matmul(out=ps_B[:], lhsT=identb[:],
                                             rhs=xlg[:, t * 512:(t + 1) * 512],
                                             start=True, stop=False)
                            nc.tensor.matmul(out=ps_B[:],
                                             lhsT=ea_w[:, t * 128:(t + 1) * 128],
                                             rhs=We2[:], start=False, stop=False)
                            nc.tensor.matmul(out=ps_B[:], lhsT=ohT[:],
                                             rhs=xr2_own[:, w * 512:(w + 1) * 512],
                                             start=False, stop=True)
                            s = fat.tile([128, 512], BF16, name=f"s2_{w}_{t}", tag="s")
                            nc.scalar.activation(out=s[:], in_=ps_B[:],
                                                 func=AF.Prelu, alpha=0.2)
                            logits_we(s, att2b, 4, we_w[:, t * 4:(t + 1) * 4], w, t, 2)

                        # ---- pass B ----
                        ps_O2 = psG.tile([128, 512], F32, name=f"psO2_{w}", tag="G")
                        ps_z2 = psW.tile([128, 4], F32, name=f"psz2_{w}", tag="Wz")
                        for t in range(T_w):
                            re = fat.tile([128, 516], BF16, name=f"re2_{w}_{t}", tag="re")
                            for h in range(4):
                                nc.vector.tensor_scalar(
                                    out=re[:, h * 128:(h + 1) * 128],
                                    in0=xlg[:, t * 512 + h * 128:t * 512 + (h + 1) * 128],
                                    scalar1=we_w[:, t * 4 + h:t * 4 + h + 1],
                                    scalar2=None, op0=OP.mult)
                            nc.vector.tensor_copy(out=re[:, 512:516],
                                                  in_=we_w[:, t * 4:(t + 1) * 4])
                            oh_sl = ohs[:, t * 128:(t + 1) * 128]
                            nc.tensor.matmul(out=ps_O2[:], lhsT=oh_sl, rhs=re[:, 0:512],
                                             start=(t == 0), stop=(t == T_w - 1))
                            nc.tensor.matmul(out=ps_z2[:], lhsT=oh_sl, rhs=re[:, 512:516],
                                             start=(t == 0), stop=(t == T_w - 1))

                        # ---- flush: mean over heads + elu + xl3/xr3 builds ----
                        rz = rz_from(ps_z2[:], 4, w, 2, quarter=True)
                        m4 = io.tile([128, 512], F32, name=f"m4_{w}", tag="m4")
                        for h in range(4):
                            nc.vector.tensor_scalar(out=m4[:, h * 128:(h + 1) * 128],
                                                    in0=ps_O2[:, h * 128:(h + 1) * 128],
                                                    scalar1=rz[:, h:h + 1],
                                                    scalar2=None, op0=OP.mult)
                        m01 = io.tile([128, 128], F32, name=f"m01_{w}", tag="m01")
                        nc.vector.tensor_tensor(out=m01[:], in0=m4[:, 0:128],
                                                in1=m4[:, 128:256], op=OP.add)
                        m23 = io.tile([128, 128], F32, name=f"m23_{w}", tag="m23")
                        nc.vector.tensor_tensor(out=m23[:], in0=m4[:, 256:384],
                                                in1=m4[:, 384:512], op=OP.add)
                        a2 = io.tile([128, 128], F32, name=f"a2_{w}", tag="a2")
                        nc.vector.tensor_tensor(out=a2[:], in0=m01[:], in1=m23[:], op=OP.add)
                        nc.vector.tensor_tensor(out=a2[:], in0=a2[:], in1=const2b[:], op=OP.add)
                        h2f = elu_of(a2, 128, w, 2)
                        if _DEBUG:
                            nc.sync.dma_start(out=h2_dbg.ap()[w * 128:(w + 1) * 128, :],
                                              in_=h2f[:])
                        h2b = io.tile([128, 128], BF16, name=f"h2b_{w}", tag="h2b")
                        nc.vector.tensor_copy(out=h2b[:], in_=h2f[:])
                        h2T = transposeb(h2b[:], f"h2_{w}", "act")
                        ps_x3 = psS.tile([128, 128], F32, name=f"psx3_{w}", tag="S")
                        nc.tensor.matmul(out=ps_x3[:], lhsT=h2T[:], rhs=Wl3[:],
                                         start=True, stop=True)
                        xl3_w = io.tile([128, 128], F32, name=f"xl3_{w}", tag="xl3")
                        nc.scalar.copy(out=xl3_w[:], in_=ps_x3[:])
                        nc.sync.dma_start(out=xl3_bounce[w * 128:(w + 1) * 128, :], in_=xl3_w[:])
                        ps_r3 = psS.tile([128, 128], F32, name=f"psr3_{w}", tag="S")
                        nc.tensor.matmul(out=ps_r3[:], lhsT=h2T[:], rhs=Wr3[:],
                                         start=True, stop=False)
                        nc.tensor.matmul(out=ps_r3[:], lhsT=ones1[:], rhs=biasr3[:],
                                         start=False, stop=True)
                        nc.scalar.copy(out=xr3_own[:, w * 128:(w + 1) * 128], in_=ps_r3[:])

                # =========================================================
                # LAYER 3 (H=1) + head
                # =========================================================
                if _LAYERS >= 3:
                    if _NOCOLL or _NOCOLL3:
                        nc.sync.dma_start(out=xl3_full[0:NPAD, :], in_=xl3_bounce[:])
                    else:
                        nc.gpsimd.collective_compute(
                            "AllGather", OP.bypass,
                            replica_groups=[list(range(NC_CORES))],
                            ins=[xl3_bounce[:]], outs=[xl3_full[:]])

                    for w in range(_WLIM):
                        drel_w, ea_w = window_meta(w, 3)
                        idx_w = io.tile([128, T_w], I32, name=f"ix3_{w}", tag="ix")
                        nc.sync.dma_start(out=idx_w[:],
                                          in_=src23_d.ap()[w * 128:(w + 1) * 128, :])
                        xga3 = big.tile([128, EPW], F32, name=f"xga3_{w}", tag="gxa3")
                        for tt in range(T_w):
                            nc.gpsimd.indirect_dma_start(
                                out=xga3[:, tt * 128:(tt + 1) * 128], out_offset=None,
                                in_=xl3_full[:],
                                in_offset=bass.IndirectOffsetOnAxis(
                                    ap=idx_w[:, tt:tt + 1], axis=0))
                        ohs = big.tile([128, EPW], BF16, name=f"ohs3_{w}", tag="ohs")
                        we_w = io.tile([128, T_w], F32, name=f"wew3_{w}", tag="wew")

                        # ---- pass A ----
                        for t in range(T_w):
                            onehot_to(ohs, drel_w, t, 3, w)
                            ohT = transposeb(ohs[:, t * 128:(t + 1) * 128],
                                             f"o3_{w}_{t}", "act")
                            ps_B = psS.tile([128, 128], F32, name=f"psB3_{w}_{t}", tag="S")
                            nc.tensor.matmul(out=ps_B[:],
                                             lhsT=ea_w[:, t * 128:(t + 1) * 128],
                                             rhs=We3[:], start=True, stop=False)
                            nc.tensor.matmul(out=ps_B[:], lhsT=ohT[:],
                                             rhs=xr3_own[:, w * 128:(w + 1) * 128],
                                             start=False, stop=True)
                            sp = fat.tile([128, 128], BF16, name=f"sp3_{w}_{t}", tag="sp")
                            nc.vector.tensor_tensor(out=sp[:],
                                                    in0=xga3[:, t * 128:(t + 1) * 128],
                                                    in1=ps_B[:], op=OP.add)
                            s = fat.tile([128, 128], BF16, name=f"s3_{w}_{t}", tag="s")
                            nc.scalar.activation(out=s[:], in_=sp[:],
                                                 func=AF.Prelu, alpha=0.2)
                            logits_we(s, att3b, 1, we_w[:, t:t + 1], w, t, 3)

                        # ---- pass B ----
                        ps_O3 = psG.tile([128, 128], F32, name=f"psO3_{w}", tag="G")
                        ps_z3 = psW.tile([128, 1], F32, name=f"psz3_{w}", tag="Wz")
                        for t in range(T_w):
                            At = fat.tile([128, 129], BF16, name=f"At3_{w}_{t}", tag="re")
                            nc.vector.tensor_scalar(
                                out=At[:, 0:128], in0=xga3[:, t * 128:(t + 1) * 128],
                                scalar1=we_w[:, t:t + 1], scalar2=None, op0=OP.mult)
                            nc.vector.tensor_copy(out=At[:, 128:129], in_=we_w[:, t:t + 1])
                            oh_sl = ohs[:, t * 128:(t + 1) * 128]
                            nc.tensor.matmul(out=ps_O3[:], lhsT=oh_sl, rhs=At[:, 0:128],
                                             start=(t == 0), stop=(t == T_w - 1))
                            nc.tensor.matmul(out=ps_z3[:], lhsT=oh_sl, rhs=At[:, 128:129],
                                             start=(t == 0), stop=(t == T_w - 1))

                        # ---- flush + head ----
                        rz = rz_from(ps_z3[:], 1, w, 3)
                        o3 = io.tile([128, 128], F32, name=f"o3_{w}", tag="o3")
                        nc.vector.tensor_scalar(out=o3[:], in0=ps_O3[:],
                                                scalar1=rz[:, 0:1], scalar2=None,
                                                op0=OP.mult)
                        nc.vector.tensor_tensor(out=o3[:], in0=o3[:], in1=const3b[:],
                                                op=OP.add)
                        h3f = elu_of(o3, 128, w, 3)
                        if _DEBUG:
                            nc.sync.dma_start(out=h3_dbg.ap()[w * 128:(w + 1) * 128, :],
                                              in_=h3f[:])
                        h3b = io.tile([128, 128], BF16, name=f"h3b_{w}", tag="h3b")
                        nc.vector.tensor_copy(out=h3b[:], in_=h3f[:])
                        h3T = transposeb(h3b[:], f"h3_{w}", "act")
                        ps_c1 = psS.tile([128, 64], F32, name=f"psc1_{w}", tag="S")
                        nc.tensor.matmul(out=ps_c1[:], lhsT=h3T[:], rhs=Wc1[:],
                                         start=True, stop=False)
                        nc.tensor.matmul(out=ps_c1[:], lhsT=ones1[:], rhs=bc1[:],
                                         start=False, stop=True)
                        a1 = io.tile([128, 64], F32, name=f"a1_{w}", tag="a1")
                        nc.scalar.copy(out=a1[:], in_=ps_c1[:])
                        c1 = elu_of(a1, 64, w, 4)
                        c1b = io.tile([128, 64], BF16, name=f"c1b_{w}", tag="c1b")
                        nc.vector.tensor_copy(out=c1b[:], in_=c1[:])
                        pt = psT.tile([128, 128], BF16, name=f"ptc_{w}", tag="pt")
                        nc.tensor.transpose(out=pt[0:64, :], in_=c1b[:], identity=identb[:])
                        c1T = io.tile([64, 128], BF16, name=f"c1T_{w}", tag="c1T")
                        nc.scalar.copy(out=c1T[:], in_=pt[0:64, :])
                        ps_f = psS.tile([128, 3], F32, name=f"psf_{w}", tag="S")
                        nc.tensor.matmul(out=ps_f[:], lhsT=c1T[:], rhs=Wc2[:],
                                         start=True, stop=False)
                        nc.tensor.matmul(out=ps_f[:], lhsT=ones1[:], rhs=bc2[:],
                                         start=False, stop=True)
                        fo = io.tile([128, 3], F32, name=f"fo_{w}", tag="fo")
                        nc.scalar.copy(out=fo[:], in_=ps_f[:])
                        nc.sync.dma_start(out=out_d.ap()[w * 128:(w + 1) * 128, :], in_=fo[:])

    nc.compile()
    return nc


# ----------------------------------------------------------------------------
# entry point
# ----------------------------------------------------------------------------

_cache = {}


def kernel(**inputs):
    x = np.ascontiguousarray(np.asarray(inputs["x"], dtype=np.float32))
    edge_index = np.asarray(inputs["edge_index"]).astype(np.int64)
    edge_attr = np.ascontiguousarray(np.asarray(inputs["edge_attr"], dtype=np.float32))

    T_w, EPW, src23, drel, eaT, xga = _host_prep(x, edge_index, edge_attr)

    f32 = lambda a: np.ascontiguousarray(np.asarray(a, dtype=np.float32))
    Wl1, bl1, Wr1, br1 = map(f32, (inputs["Wl1"], inputs["bl1"], inputs["Wr1"], inputs["br1"]))
    We1, att1, bo1 = map(f32, (inputs["We1"], inputs["att1"], inputs["bo1"]))
    Wl2, bl2, Wr2, br2 = map(f32, (inputs["Wl2"], inputs["bl2"], inputs["Wr2"], inputs["br2"]))
    We2, att2, bo2 = map(f32, (inputs["We2"], inputs["att2"], inputs["bo2"]))
    Wl3, bl3, Wr3, br3 = map(f32, (inputs["Wl3"], inputs["bl3"], inputs["Wr3"], inputs["br3"]))
    We3, att3, bo3 = map(f32, (inputs["We3"], inputs["att3"], inputs["bo3"]))
    Wres, bres = map(f32, (inputs["Wres"], inputs["bres"]))
    Wc1, bc1, Wc2, bc2 = map(f32, (inputs["Wc1"], inputs["bc1"], inputs["Wc2"], inputs["bc2"]))

    if T_w not in _cache:
        _cache[T_w] = _build_program(T_w)
    nc = _cache[T_w]

    bf = lambda a: np.ascontiguousarray(np.asarray(a).astype(bfnp))
    common = {
        "iotarb": bf(np.tile(np.arange(128, dtype=np.float32).reshape(1, 128),
                             (128, 1))),
        "Wl1": bf(Wl1), "Wr1": bf(Wr1), "Wres": bf(Wres), "We1": bf(We1),
        "att1b": bf(np.tile(att1.reshape(1, 1024), (128, 1))),
        "Wl2c": bf(_chunks_for_rhs(Wl2)), "Wr2c": bf(_chunks_for_rhs(Wr2)),
        "We2": bf(We2), "att2b": bf(np.tile(att2.reshape(1, 512), (128, 1))),
        "Wl3": bf(Wl3), "Wr3": bf(Wr3), "We3": bf(We3),
        "att3b": bf(np.tile(att3.reshape(1, 128), (128, 1))),
        "Wc1": bf(Wc1), "Wc2": bf(Wc2),
        "biasr1": bf((br1 + bl1).reshape(1, 1024)),
        "const1": bf((bl1 + bo1 + bres).reshape(1, 1024)),
        "biasr2": bf((br2 + bl2).reshape(1, 512)),
        "const2b": np.tile((bl2.reshape(4, 128).mean(0) + bo2).reshape(1, 128),
                           (128, 1)).astype(np.float32),
        "biasr3": bf((br3 + bl3).reshape(1, 128)),
        "const3b": np.tile((bl3 + bo3).reshape(1, 128), (128, 1)).astype(np.float32),
        "bc1": bf(bc1.reshape(1, 64)), "bc2": bf(bc2.reshape(1, 3)),
    }

    def tilemajor(a, dt=np.float32):
        return np.ascontiguousarray(
            a.reshape(W, T_w, 128).transpose(0, 2, 1).reshape(W * 128, T_w)
            .astype(dt))

    in_maps = []
    for c in range(NC_CORES):
        m = dict(common)
        m["x_ownT"] = bf(_pad_own(x, c).T)
        m["xga_t"] = np.ascontiguousarray(xga[c])
        m["src23i"] = tilemajor(src23[c], np.int32)
        m["drel"] = tilemajor(drel[c])
        m["eaT"] = bf(eaT[c].reshape(W * ED, EPW))
        in_maps.append(m)

    kernel._last_in_maps = in_maps
    res = run_bass_kernel_spmd(nc, in_maps, core_ids=list(range(NC_CORES)), trace=False)
    out = np.concatenate([res.results[c]["out_o"][:PER] for c in range(NC_CORES)], axis=0)
    if _DEBUG:
        kernel._last_results = res.results
    return out.astype(np.float32)
